# revision 8
# baseline (speedup 1.0000x reference)
"""MACE layer on 8 TRN2 NeuronCores.

Sharding: nodes partitioned into 8 contiguous ranges (1250 each); edges
assigned to the core owning their receiver, sorted by receiver. Scatter is
core-local; sender features are staged per-edge by the host (halo-free).
Edges are packed into windows of <=32 consecutive receiver nodes and <=512
edges (4 tiles of 128); the per-window aggregate accumulates in PSUM via
one-hot scatter matmuls. Node phase runs channel-major over slot space;
host maps slots back to nodes at the end.
"""
import numpy as np
import ml_dtypes

import concourse.bass as bass
import concourse.mybir as mybir
from concourse.tile import TileContext
import concourse.tile as tile_mod
from concourse.bass_utils import run_bass_kernel_spmd

BF = ml_dtypes.bfloat16
N, E, C, MLPD, F = 10000, 160000, 128, 64, 128
AVG = 16.0
NCORE = 8
NLOC = N // NCORE
WN, WE, TPW = 32, 512, 4  # nodes/window cap, edges/window, tiles/window

# ---- patch: this walrus build rejects >2 sem waits on one instruction ----
_orig_drain = tile_mod.TileContext._drain_and_barrier


def _patched_drain(self, tick_clock, wait_clock):
    nc = self.nc
    from concourse.vector_clock import ScopedClock
    drain_inst = nc.sync.drain()
    wait_clock.add_sem_waits(
        drain_inst.ins, ScopedClock({None: tick_clock.global_clock})
    )
    si = drain_inst.ins.sync_info
    waits = list(si.on_wait or [])
    if len(waits) > 1:
        del si.on_wait[:]
        bb = nc.cur_bb.bb
        assert bb.instructions[-1] is drain_inst.ins
        bb.instructions.pop()
        for w in waits:
            d2 = nc.sync.drain()
            d2si = d2.ins.sync_info
            if d2si is None:
                d2.ins.sync_info = mybir.SyncInfo(on_wait=[w], on_update=[])
            else:
                d2si.on_wait.append(w)
        bb.instructions.append(drain_inst.ins)
    nc.all_engine_barrier()
    assert self.sems is not None
    popped = nc._tile_sem_poison_stack.pop()
    assert popped is self._sem_poison
    nc.clear_and_free_semaphores(list(self.sems.allocated().values()))
    nc.all_engine_barrier()


tile_mod.TileContext._drain_and_barrier = _patched_drain


def _split_waits(nc, maxw=1):
    """This walrus build caps sync waits per instruction; hoist excess waits
    onto same-engine no-ops placed just before the instruction."""
    cnt = 0
    for f in nc.m.functions:
        for bb in f.blocks:
            out = []
            for inst in bb.instructions:
                si = getattr(inst, "sync_info", None)
                if si is not None and si.on_wait and len(si.on_wait) > maxw:
                    waits = list(si.on_wait)
                    del si.on_wait[:]
                    si.on_wait.extend(waits[:maxw])
                    for w in waits[maxw:]:
                        nop = mybir.InstNoOp(name=f"wsplit_{cnt}", ins=[], outs=[])
                        cnt += 1
                        nop.engine = inst.engine
                        nop.sync_info = mybir.SyncInfo(on_wait=[w], on_update=[])
                        nc.register_instruction(nop, overwrite=True)
                        out.append(nop)
                out.append(inst)
            bb.instructions[:] = out

f32, bf16 = mybir.dt.float32, mybir.dt.bfloat16
AF = mybir.ActivationFunctionType


def build(G):
    """One SPMD program; G windows per core."""
    nc = bass.Bass()
    T = G * WE          # padded edges per core
    S = G * WN          # slots per core
    NCH = G // 8        # node/edge chunks of 8 windows
    CH_E = 8 * WE       # 4096 edges per chunk

    din = {}
    for nm, sh, dt in [
        ("nfs", [C, T], bf16), ("ef", [C, T], bf16), ("lenr", [1, T], bf16),
        ("oh", [C, (T // 128) * 32], bf16), ("y8", [C, (T // 128) * 8], f32),
        ("wup", [C, C], bf16), ("wr1a", [C, MLPD], bf16),
        ("wr1b", [1, MLPD], bf16), ("wr2", [MLPD, 3 * C], bf16),
        ("wl0", [C, C], bf16), ("wl1", [C, C], bf16), ("wl2", [C, C], bf16),
        ("p0", [C, C], bf16), ("p1", [C, C], bf16), ("w1", [C, MLPD], bf16),
        ("w2", [MLPD, F], bf16), ("w3", [MLPD, 1], bf16), ("wvb", [C, 1], bf16),
        ("w0t", [C, 8], f32), ("w1t", [C, 6], f32),
        ("br1", [MLPD, 1], f32), ("b1", [MLPD, 1], f32),
    ]:
        din[nm] = nc.dram_tensor(nm, sh, dt, kind="ExternalInput")
    o_h0 = nc.dram_tensor("o_h0", [C, S], f32, kind="ExternalOutput")
    o_h1 = nc.dram_tensor("o_h1", [C, 3 * S], f32, kind="ExternalOutput")
    o_scal = nc.dram_tensor("o_scal", [F, S], f32, kind="ExternalOutput")
    o_vec = nc.dram_tensor("o_vec", [1, 3 * S], f32, kind="ExternalOutput")

    with TileContext(nc) as tc:
        with tc.tile_pool(name="wpool", bufs=1) as wp, \
             tc.tile_pool(name="stream", bufs=2) as sp, \
             tc.tile_pool(name="work", bufs=3) as kp, \
             tc.tile_pool(name="aggbuf", bufs=1) as ap:

            W = {}
            for nm in ("wup", "wr1a", "wr1b", "wr2", "wl0", "wl1", "wl2",
                       "p0", "p1", "w1", "w2", "w3", "wvb", "w0t", "w1t",
                       "br1", "b1"):
                t = wp.tile(din[nm].shape, din[nm].dtype, tag=nm)
                nc.sync.dma_start(out=t[:], in_=din[nm][:])
                W[nm] = t

            agg0 = ap.tile([C, G * 32], bf16, tag="agg0")
            agg1 = ap.tile([C, G * 96], bf16, tag="agg1")
            agg2 = ap.tile([C, G * 160], bf16, tag="agg2")

            # ---------------- edge phase ----------------
            epools = [
                tc.tile_pool(name="psEA", bufs=1, space="PSUM"),
                tc.tile_pool(name="psEB", bufs=2, space="PSUM")]
            pA, pB = (p.__enter__() for p in epools)
            pA, pB = epools[0].__enter__() if False else pA, pB
            for ch in range(NCH):
                e0 = ch * CH_E
                nfs_c = sp.tile([C, CH_E], bf16, tag="nfs")
                ef_c = sp.tile([C, CH_E], bf16, tag="ef")
                len_c = sp.tile([1, CH_E], bf16, tag="len")
                oh_c = sp.tile([C, 8 * TPW * 32], bf16, tag="oh")
                y8_c = sp.tile([C, 8 * TPW * 8], f32, tag="y8")
                nc.sync.dma_start(out=nfs_c[:], in_=din["nfs"][:, e0:e0 + CH_E])
                nc.sync.dma_start(out=ef_c[:], in_=din["ef"][:, e0:e0 + CH_E])
                nc.sync.dma_start(out=len_c[:], in_=din["lenr"][:, e0:e0 + CH_E])
                nc.sync.dma_start(
                    out=oh_c[:], in_=din["oh"][:, ch * 8 * TPW * 32:(ch + 1) * 8 * TPW * 32])
                nc.sync.dma_start(
                    out=y8_c[:], in_=din["y8"][:, ch * 8 * TPW * 8:(ch + 1) * 8 * TPW * 8])

                for gw in range(8):
                    g = ch * 8 + gw
                    a0p = pA.tile([C, 32], f32, tag="a0p")
                    a1p = pA.tile([C, 96], f32, tag="a1p")
                    a2p = pA.tile([C, 160], f32, tag="a2p")
                    for j in range(TPW):
                        t_in_ch = gw * TPW + j
                        es = t_in_ch * 128
                        hs_ps = pA.tile([128, C], f32, tag="hs_ps")
                        nc.tensor.matmul(
                            out=hs_ps[:], lhsT=nfs_c[:, es:es + 128],
                            rhs=W["wup"][:], start=True, stop=True)
                        hs_sb = kp.tile([128, C], bf16, tag="hs_sb")
                        nc.scalar.copy(out=hs_sb[:], in_=hs_ps[:])

                        z_ps = pB.tile([MLPD, 128], f32, tag="z_ps")
                        nc.tensor.matmul(
                            out=z_ps[:], lhsT=W["wr1a"][:],
                            rhs=ef_c[:, es:es + 128], start=True, stop=False)
                        nc.tensor.matmul(
                            out=z_ps[:], lhsT=W["wr1b"][:],
                            rhs=len_c[:, es:es + 128], start=False, stop=True)
                        z_sb = kp.tile([MLPD, 128], bf16, tag="z_sb")
                        nc.scalar.activation(
                            out=z_sb[:], in_=z_ps[:], func=AF.Silu,
                            bias=W["br1"][:, 0:1], scale=1.0)

                        r_ps = pB.tile([128, 3 * C], f32, tag="r_ps")
                        nc.tensor.matmul(
                            out=r_ps[:], lhsT=z_sb[:], rhs=W["wr2"][:],
                            start=True, stop=True)
                        r_sb = kp.tile([128, 3 * C], bf16, tag="r_sb")
                        nc.scalar.copy(out=r_sb[:], in_=r_ps[:])

                        w_sb = kp.tile([128, 3 * C], bf16, tag="w_sb")
                        for l in range(3):
                            nc.vector.tensor_mul(
                                out=w_sb[:, l * C:(l + 1) * C],
                                in0=hs_sb[:], in1=r_sb[:, l * C:(l + 1) * C])

                        ohs = oh_c[:, t_in_ch * 32:(t_in_ch + 1) * 32]
                        a_sb = kp.tile([128, 8 * 32], bf16, tag="a_sb")
                        for k in range(8):
                            nc.vector.tensor_scalar_mul(
                                a_sb[:, k * 32:(k + 1) * 32], ohs,
                                y8_c[:, t_in_ch * 8 + k:t_in_ch * 8 + k + 1])

                        st, sp_ = (j == 0), (j == TPW - 1)
                        nc.tensor.matmul(out=a0p[:], lhsT=w_sb[:, 0:C],
                                         rhs=ohs, start=st, stop=sp_)
                        nc.tensor.matmul(out=a1p[:], lhsT=w_sb[:, C:2 * C],
                                         rhs=a_sb[:, 0:96], start=st, stop=sp_)
                        nc.tensor.matmul(out=a2p[:], lhsT=w_sb[:, 2 * C:3 * C],
                                         rhs=a_sb[:, 96:256], start=st, stop=sp_)
                    nc.scalar.copy(out=agg0[:, g * 32:(g + 1) * 32], in_=a0p[:])
                    nc.scalar.copy(out=agg1[:, g * 96:(g + 1) * 96], in_=a1p[:])
                    nc.scalar.copy(out=agg2[:, g * 160:(g + 1) * 160], in_=a2p[:])

            for p in reversed(epools):
                p.__exit__(None, None, None)
            npools = [
                tc.tile_pool(name="psNA", bufs=2, space="PSUM"),
                tc.tile_pool(name="psNB", bufs=2, space="PSUM")]
            pA = npools[0].__enter__()
            pB = npools[1].__enter__()

            # ---------------- node phase ----------------
            SQ3 = float(np.sqrt(3.0))
            for ch in range(NCH):
                cs, c3s, c5s = ch * 256, ch * 768, ch * 1280
                sb_s = kp.tile([C, 256], bf16, tag="sb_s")
                sb_v = kp.tile([C, 768], bf16, tag="sb_v")
                sb_t = kp.tile([C, 1280], bf16, tag="sb_t")
                mmjobs = [(W["wl0"], agg0, cs, 256, sb_s),
                          (W["wl1"], agg1, c3s, 768, sb_v),
                          (W["wl2"], agg2, c5s, 1280, sb_t)]
                for wmat, asrc, base, tot, dst in mmjobs:
                    for lo in range(0, tot, 512):
                        hi = min(lo + 512, tot)
                        nps = pA.tile([C, hi - lo], f32, tag="nps",
                                      name=f"nps_{ch}_{base}_{lo}")
                        nc.tensor.matmul(out=nps[:], lhsT=wmat[:],
                                         rhs=asrc[:, base + lo:base + hi],
                                         start=True, stop=True)
                        nc.scalar.copy(out=dst[:, lo:hi], in_=nps[:])

                v4 = sb_v[:].rearrange("p (g x s) -> p g x s", x=3, s=32)
                t4 = sb_t[:].rearrange("p (g x s) -> p g x s", x=5, s=32)
                vx, vy, vz = v4[:, :, 0], v4[:, :, 1], v4[:, :, 2]
                ta, tb, tc_, td, te = (t4[:, :, i] for i in range(5))

                def mk(tag, w=256):
                    return kp.tile([C, w], bf16, tag=tag, name=f"nd_{tag}_{ch}")

                def mul(o, x, y):
                    nc.vector.tensor_mul(out=o, in0=x, in1=y)

                def add(o, x, y):
                    nc.vector.tensor_add(out=o, in0=x, in1=y)

                sq = mk("sq", 768)
                mul(sq[:], sb_v[:], sb_v[:])
                s4 = sq[:].rearrange("p (g x s) -> p g x s", x=3, s=32)
                v2 = mk("v2")
                add(v2[:].rearrange("p (g s) -> p g s", s=32), s4[:, :, 0], s4[:, :, 1])
                add(v2[:].rearrange("p (g s) -> p g s", s=32),
                    v2[:].rearrange("p (g s) -> p g s", s=32), s4[:, :, 2])
                sq5 = mk("sq5", 1280)
                mul(sq5[:], sb_t[:], sb_t[:])
                q4 = sq5[:].rearrange("p (g x s) -> p g x s", x=5, s=32)
                t2 = mk("t2")
                t2r = t2[:].rearrange("p (g s) -> p g s", s=32)
                add(t2r, q4[:, :, 0], q4[:, :, 1])
                add(t2r, t2r, q4[:, :, 2])
                add(t2r, t2r, q4[:, :, 3])
                add(t2r, t2r, q4[:, :, 4])

                c3 = mk("c3")
                c3r = c3[:].rearrange("p (g s) -> p g s", s=32)
                nc.vector.tensor_scalar_mul(c3r, tc_, 1.0 / SQ3)
                p_ = mk("p_")
                pr = p_[:].rearrange("p (g s) -> p g s", s=32)
                nc.vector.tensor_tensor(out=pr, in0=te, in1=c3r,
                                        op=mybir.AluOpType.subtract)
                qq = mk("qq")
                qr = qq[:].rearrange("p (g s) -> p g s", s=32)
                add(qr, te, c3r)
                nc.vector.tensor_scalar_mul(qr, qr, -1.0)
                r2 = mk("r2")
                r2r = r2[:].rearrange("p (g s) -> p g s", s=32)
                nc.vector.tensor_scalar_mul(r2r, c3r, 2.0)

                Tv = mk("Tv", 768)
                Tv4 = Tv[:].rearrange("p (g x s) -> p g x s", x=3, s=32)
                tmp = mk("tmp")
                tmpr = tmp[:].rearrange("p (g s) -> p g s", s=32)
                for xi, (m0, m1, m2) in enumerate(
                        [(pr, ta, td), (ta, qr, tb), (td, tb, r2r)]):
                    dst = Tv4[:, :, xi]
                    mul(dst, m0, vx)
                    mul(tmpr, m1, vy)
                    add(dst, dst, tmpr)
                    mul(tmpr, m2, vz)
                    add(dst, dst, tmpr)
                vTv = mk("vTv")
                vr = vTv[:].rearrange("p (g s) -> p g s", s=32)
                mul(vr, Tv4[:, :, 0], vx)
                mul(tmpr, Tv4[:, :, 1], vy)
                add(vr, vr, tmpr)
                mul(tmpr, Tv4[:, :, 2], vz)
                add(vr, vr, tmpr)

                s2 = mk("s2")
                mul(s2[:], sb_s[:], sb_s[:])
                s3 = mk("s3")
                mul(s3[:], s2[:], sb_s[:])
                sv2 = mk("sv2")
                mul(sv2[:], sb_s[:], v2[:])
                st2 = mk("st2")
                mul(st2[:], sb_s[:], t2[:])

                B0 = mk("B0")
                acc = mk("acc")
                terms0 = [sb_s[:], s2[:], v2[:], t2[:], s3[:], sv2[:], st2[:], vTv[:]]
                for k, trm in enumerate(terms0):
                    dst = B0[:] if k == 0 else acc[:]
                    nc.vector.tensor_scalar_mul(dst, trm, W["w0t"][:, k:k + 1])
                    if k:
                        add(B0[:], B0[:], acc[:])

                B1 = mk("B1", 768)
                B14 = B1[:].rearrange("p (g x s) -> p g x s", x=3, s=32)
                accr = acc[:].rearrange("p (g s) -> p g s", s=32)
                sv = mk("sv", 768)
                sv4 = sv[:].rearrange("p (g x s) -> p g x s", x=3, s=32)
                for xi, vv in enumerate((vx, vy, vz)):
                    mul(sv4[:, :, xi], sb_s[:].rearrange("p (g s) -> p g s", s=32), vv)
                for xi in range(3):
                    vv = v4[:, :, xi]
                    dst = B14[:, :, xi]
                    t1l = [vv, sv4[:, :, xi], Tv4[:, :, xi], None, None, None]
                    nc.vector.tensor_scalar_mul(dst, vv, W["w1t"][:, 0:1])
                    for k, trm in [(1, sv4[:, :, xi]), (2, Tv4[:, :, xi])]:
                        nc.vector.tensor_scalar_mul(accr, trm, W["w1t"][:, k:k + 1])
                        add(dst, dst, accr)
                    sr = sb_s[:].rearrange("p (g s) -> p g s", s=32)
                    mul(tmpr, s2[:].rearrange("p (g s) -> p g s", s=32), vv)
                    nc.vector.tensor_scalar_mul(tmpr, tmpr, W["w1t"][:, 3:4])
                    add(dst, dst, tmpr)
                    mul(tmpr, v2[:].rearrange("p (g s) -> p g s", s=32), vv)
                    nc.vector.tensor_scalar_mul(tmpr, tmpr, W["w1t"][:, 4:5])
                    add(dst, dst, tmpr)
                    mul(tmpr, sr, Tv4[:, :, xi])
                    nc.vector.tensor_scalar_mul(tmpr, tmpr, W["w1t"][:, 5:6])
                    add(dst, dst, tmpr)

                h0_f = kp.tile([C, 256], f32, tag="h0_f")
                h0_sb = kp.tile([C, 256], bf16, tag="h0_sb")
                h1_f = kp.tile([C, 768], f32, tag="h1_f")
                h1_sb = kp.tile([C, 768], bf16, tag="h1_sb")
                ops = pB.tile([C, 256], f32, tag="ops", name=f"oph0_{ch}")
                nc.tensor.matmul(out=ops[:], lhsT=W["p0"][:], rhs=B0[:],
                                 start=True, stop=True)
                nc.scalar.copy(out=h0_f[:], in_=ops[:])
                nc.scalar.copy(out=h0_sb[:], in_=ops[:])
                for q in range(2):
                    lo, hi = q * 512, min((q + 1) * 512, 768)
                    ops = pB.tile([C, hi - lo], f32, tag="ops", name=f"oph1_{ch}_{q}")
                    nc.tensor.matmul(out=ops[:], lhsT=W["p1"][:],
                                     rhs=B1[:, lo:hi], start=True, stop=True)
                    nc.scalar.copy(out=h1_f[:, lo:hi], in_=ops[:])
                    nc.scalar.copy(out=h1_sb[:, lo:hi], in_=ops[:])
                nc.sync.dma_start(out=o_h0[:, cs:cs + 256], in_=h0_f[:])
                nc.sync.dma_start(out=o_h1[:, c3s:c3s + 768], in_=h1_f[:])

                z2_ps = pB.tile([MLPD, 256], f32, tag="ops", name=f"opz_{ch}")
                nc.tensor.matmul(out=z2_ps[:], lhsT=W["w1"][:], rhs=h0_sb[:],
                                 start=True, stop=True)
                z2_sb = kp.tile([MLPD, 256], bf16, tag="z2_sb")
                nc.scalar.activation(out=z2_sb[:], in_=z2_ps[:], func=AF.Silu,
                                     bias=W["b1"][:, 0:1], scale=1.0)

                sc_ps = pB.tile([F, 256], f32, tag="ops", name=f"opsc_{ch}")
                nc.tensor.matmul(out=sc_ps[:], lhsT=W["w2"][:], rhs=z2_sb[:],
                                 start=True, stop=True)
                sc_f = kp.tile([F, 256], f32, tag="sc_f")
                nc.scalar.copy(out=sc_f[:], in_=sc_ps[:])
                nc.sync.dma_start(out=o_scal[:, cs:cs + 256], in_=sc_f[:])

                gt_ps = pB.tile([1, 256], f32, tag="ops", name=f"opgt_{ch}")
                nc.tensor.matmul(out=gt_ps[:], lhsT=W["w3"][:], rhs=z2_sb[:],
                                 start=True, stop=True)
                gt_sb = kp.tile([1, 256], f32, tag="gt_sb")
                nc.scalar.copy(out=gt_sb[:], in_=gt_ps[:])
                vv_f = kp.tile([1, 768], f32, tag="vv_f")
                for q in range(2):
                    lo, hi = q * 512, min((q + 1) * 512, 768)
                    ops = pB.tile([1, hi - lo], f32, tag="ops", name=f"opvv_{ch}_{q}")
                    nc.tensor.matmul(out=ops[:], lhsT=W["wvb"][:],
                                     rhs=h1_sb[:, lo:hi], start=True, stop=True)
                    nc.scalar.copy(out=vv_f[:, lo:hi], in_=ops[:])
                vv_sb = kp.tile([1, 768], f32, tag="vv_sb")
                vvp4 = vv_f[:].rearrange("p (g x s) -> p g x s", x=3, s=32)
                vvs4 = vv_sb[:].rearrange("p (g x s) -> p g x s", x=3, s=32)
                gtr = gt_sb[:].rearrange("p (g s) -> p g s", s=32)
                for xi in range(3):
                    nc.vector.tensor_mul(out=vvs4[:, :, xi], in0=vvp4[:, :, xi], in1=gtr)
                nc.sync.dma_start(out=o_vec[:, c3s:c3s + 768], in_=vv_sb[:])
            for p in reversed(npools):
                p.__exit__(None, None, None)
    _split_waits(nc)
    return nc


def _sph_np(vec):
    u = vec / (np.linalg.norm(vec, axis=-1, keepdims=True) + 1e-9)
    x, y, z = u[:, 0], u[:, 1], u[:, 2]
    s3, s5, s15 = np.sqrt(3.0), np.sqrt(5.0), np.sqrt(15.0)
    return np.stack([s3 * x, s3 * y, s3 * z,
                     s15 * x * y, s15 * y * z, (s5 / 2) * (3 * z * z - 1),
                     s15 * x * z, (s15 / 2) * (x * x - y * y)], -1)  # [E,8]


def kernel(vectors, lengths, node_feats, edge_feats, edge_index,
           W_up, Wr1, br1, Wr2, Wl0, Wl1, Wl2, w0, w1, P0, P1,
           W1, b1, W2, W3, wv):
    vectors = np.asarray(vectors, np.float32)
    lengths = np.asarray(lengths, np.float32)
    node_feats = np.asarray(node_feats, np.float32)
    edge_feats = np.asarray(edge_feats, np.float32)
    edge_index = np.asarray(edge_index)
    snd, rcv = edge_index[0].astype(np.int64), edge_index[1].astype(np.int64)
    y8_full = _sph_np(vectors).astype(np.float32)

    # ---- per-core greedy windowing ----
    deg = np.bincount(rcv, minlength=N)
    packs = []
    Gmax = 0
    for k in range(NCORE):
        nlo = k * NLOC
        wins = []  # list of (node_lo, node_hi)
        cur_lo, cur_e = 0, 0
        for nl in range(NLOC):
            d = int(deg[nlo + nl])
            if nl > cur_lo and (nl - cur_lo >= WN or cur_e + d > WE):
                wins.append((cur_lo, nl)); cur_lo, cur_e = nl, 0
            cur_e += d
        wins.append((cur_lo, NLOC))
        packs.append(wins)
        Gmax = max(Gmax, len(wins))
    G = ((Gmax + 7) // 8) * 8
    T, S = G * WE, G * WN

    order = np.argsort(rcv, kind="stable")
    in_maps, slotmaps = [], []
    wb = {n: np.ascontiguousarray(a.astype(BF)) for n, a in [
        ("wup", W_up), ("wr1a", Wr1[:C]), ("wr1b", Wr1[C:C + 1]), ("wr2", Wr2),
        ("wl0", Wl0 / AVG), ("wl1", Wl1 / AVG), ("wl2", Wl2 / AVG),
        ("p0", P0), ("p1", P1), ("w1", W1), ("w2", W2), ("w3", W3),
        ("wvb", wv[:, None])]}
    wf = {"w0t": np.ascontiguousarray(w0.T.astype(np.float32)),
          "w1t": np.ascontiguousarray(w1.T.astype(np.float32)),
          "br1": np.ascontiguousarray(br1[:, None].astype(np.float32)),
          "b1": np.ascontiguousarray(b1[:, None].astype(np.float32))}

    for k in range(NCORE):
        nlo = k * NLOC
        lo = np.searchsorted(rcv[order], nlo)
        hi = np.searchsorted(rcv[order], nlo + NLOC)
        eidx = order[lo:hi]
        erc = rcv[eidx] - nlo
        wins = packs[k]
        perm = np.full(T, -1, np.int64)          # edge slot -> orig edge id
        lid = np.zeros(T, np.int64)              # edge slot -> window slot
        slotmap = np.full(N // NCORE, 0, np.int64)
        for g, (wlo, whi) in enumerate(wins):
            a = np.searchsorted(erc, wlo)
            b = np.searchsorted(erc, whi)
            cnt = b - a
            perm[g * WE:g * WE + cnt] = eidx[a:b]
            lid[g * WE:g * WE + cnt] = erc[a:b] - wlo
            slotmap[wlo:whi] = g * WN + np.arange(whi - wlo)
        valid = perm >= 0
        pe = np.where(valid, perm, 0)
        nfs = node_feats[snd[pe]] * valid[:, None]
        ef = edge_feats[pe] * valid[:, None]
        ln = lengths[pe, 0] * valid
        y8 = y8_full[pe] * valid[:, None]
        oh = np.zeros((T, WN), np.float32)
        oh[np.arange(T)[valid], lid[valid]] = 1.0
        nt = T // 128
        m = {
            "nfs": np.ascontiguousarray(nfs.T.astype(BF)),
            "ef": np.ascontiguousarray(ef.T.astype(BF)),
            "lenr": np.ascontiguousarray(ln[None, :].astype(BF)),
            "oh": np.ascontiguousarray(
                oh.reshape(nt, 128, WN).transpose(1, 0, 2).reshape(128, -1).astype(BF)),
            "y8": np.ascontiguousarray(
                y8.reshape(nt, 128, 8).transpose(1, 0, 2).reshape(128, -1).astype(np.float32)),
        }
        m.update(wb); m.update(wf)
        in_maps.append(m)
        slotmaps.append(slotmap)

    nc = build(G)
    res = run_bass_kernel_spmd(nc, in_maps, core_ids=list(range(NCORE)))
    kernel._last_results = res
    import os, time as _time
    if os.environ.get("KERNEL_BENCH"):
        ts = []
        for _ in range(3):
            t0 = _time.perf_counter()
            run_bass_kernel_spmd(nc, in_maps, core_ids=list(range(NCORE)))
            ts.append(_time.perf_counter() - t0)
        kernel._bench_s = min(ts)

    scal = np.empty((N, F), np.float32)
    vec = np.empty((N, 3), np.float32)
    nfo = np.empty((N, 4 * C), np.float32)
    for k in range(NCORE):
        r = res.results[k]
        sm = slotmaps[k]
        sl = slice(k * NLOC, (k + 1) * NLOC)
        scal[sl] = r["o_scal"].T[sm]
        h0 = r["o_h0"].T[sm]                     # [NLOC, C]
        h1 = r["o_h1"].reshape(C, G // 8, 8, 3, WN).transpose(1, 2, 4, 0, 3) \
                      .reshape(S, C, 3)[sm]      # [NLOC, C, 3]
        vecs = r["o_vec"].reshape(G // 8, 8, 3, WN).transpose(0, 1, 3, 2) \
                         .reshape(S, 3)[sm]
        vec[sl] = vecs
        nfo[sl, :C] = h0
        nfo[sl, C:] = h1.reshape(NLOC, 3 * C)
    return scal, vec, nfo
